# revision 31
# baseline (speedup 1.0000x reference)
"""Trainium2 Bass kernel for nn_AttentionSubsample (8-core SPMD).

Sharding: batch N=2 x 4 head-groups (3 heads each) -> 8 cores, no
collectives.  Each core computes q/k/v projections for its head group
(K/V on the stride-2 subsampled positions only), per-head attention with
softmax folded as exp -> denominator via an appended ones-column in V ->
divide, and its partial output projection in transposed layout.  The
host sums the 4 per-batch partials and adds the bias.

Layout notes:
 - The spatial stride-2 subsample of K/V equals taking even rows of the
   flattened [3136, 768] batch (196 is even), i.e. even columns of x^T.
 - All device matmuls run in bf16 (fp32 PSUM accumulation).
"""

import sys

for _p in ("/opt/trn_rl_repo",):
    if _p not in sys.path:
        sys.path.insert(0, _p)

import numpy as np
import ml_dtypes

import concourse.bass as bass  # noqa: F401  (registers engines)
import concourse.tile as tile
from concourse import bacc, mybir
from concourse.bass_utils import run_bass_kernel_spmd

BFNP = ml_dtypes.bfloat16
F32 = mybir.dt.float32
F32R = mybir.dt.float32r
BF16 = mybir.dt.bfloat16
AF = mybir.ActivationFunctionType

N, T, S, D = 2, 16, 196, 768
H, HD = 12, 64
Q = T * S              # 3136 query positions per batch
KP = T * (S // 2)      # 1568 subsampled key positions
HPG = 3                # heads per group (12 heads / 4 groups)
GD = HPG * HD          # 192 channels per head group
SC = (D // H) ** -0.5  # 0.125 attention scale
CH = 448               # q-chunk size (3136 = 7 * 448)
NCH = Q // CH          # 7
NKT = 13               # k tiles: 12 * 128 + 32
KTL = 32               # last k-tile height
NDK = D // 128         # 6 contraction tiles for the projections
N_CORES = 8

# exp groups over k-tiles: PSUM scores tile holds 3 banks (512-aligned)
EXP_GROUPS = [(0, 1, 2), (3, 4, 5), (6, 7, 8), (9, 10, 11), (12,)]

TRACE = False          # test.py flips this for profiled runs
LAST_RESULTS = {}      # exec_time_ns etc. stashed here on traced runs

_CACHE = {}


def _ksize(kt):
    return 128 if kt < NKT - 1 else KTL


def _head_pos(h):
    """(block, partition base) of head h inside the 2-block qT/kT tiles."""
    return (0, 0) if h == 0 else ((0, 64) if h == 1 else (1, 0))


def _build_nc():
    nc = bacc.Bacc(
        "TRN2", target_bir_lowering=False, debug=False, num_devices=N_CORES
    )
    # weights arrive pre-rearranged from the host in their exact SBUF
    # layouts so the loads are single contiguous DMAs
    xT = nc.dram_tensor("xT", [D, Q], BF16, kind="ExternalInput").ap()
    wq = nc.dram_tensor("wq", [128, NDK * GD], BF16, kind="ExternalInput").ap()
    wk = nc.dram_tensor("wk", [128, NDK * GD], BF16, kind="ExternalInput").ap()
    wv = nc.dram_tensor("wv", [128, NDK * GD], BF16, kind="ExternalInput").ap()
    wp = nc.dram_tensor("wp", [128, 2 * D], BF16, kind="ExternalInput").ap()
    out = nc.dram_tensor("out", [D, Q], F32, kind="ExternalOutput").ap()

    with tile.TileContext(nc) as tc:
        _body(tc, xT, wq, wk, wv, wp, out)
    nc.compile()
    return nc


def _body(tc, xT, wq, wk, wv, wp, out):
    nc = tc.nc
    with (
        tc.tile_pool(name="persist", bufs=1) as P,
        tc.tile_pool(name="es", bufs=6) as ES,
        tc.tile_pool(name="inv", bufs=2) as INV,
        tc.tile_pool(name="ot", bufs=3) as OT,
        tc.tile_pool(name="scps", bufs=2, space="PSUM") as SCPS,
        tc.tile_pool(name="numps", bufs=2, space="PSUM") as NUMPS,
    ):
        # PE warm-up scratch first so its memsets clear the vector queue
        # before the big vv memsets
        warm_w = P.tile([128, 128], BF16, tag="warm_w")
        nc.vector.memset(warm_w[:], 0.0)
        warm_x = P.tile([128, 448], BF16, tag="warm_x")
        nc.vector.memset(warm_x[:], 0.0)
        for wi in range(14):
            wps = SCPS.tile([128, 512], F32, tag="sc", name=f"warm{wi}")
            nc.tensor.matmul(
                wps[0:128, 0:448], warm_w[:], warm_x[:], start=True, stop=True
            )

        # ---- persistent SBUF tensors -------------------------------------
        wq_sb = P.tile([128, NDK * GD], BF16, tag="wq")
        nc.scalar.dma_start(wq_sb[:], wq[:])
        # x loaded chunk-major across both DMA queues so qproj c=0 starts early
        xt = P.tile([128, NDK * Q], BF16, tag="xt")
        _dmae = (nc.sync, nc.scalar)
        for c in range(NCH):
            for kt in range(NDK):
                _dmae[kt % 2].dma_start(
                    xt[:, kt * Q + c * CH : kt * Q + (c + 1) * CH],
                    xT[kt * 128 : (kt + 1) * 128, c * CH : (c + 1) * CH],
                )
        wk_sb = P.tile([128, NDK * GD], BF16, tag="wk")
        nc.sync.dma_start(wk_sb[:], wk[:])
        wv_sb = P.tile([128, NDK * GD], BF16, tag="wv")
        nc.sync.dma_start(wv_sb[:], wv[:])
        # wp: h0 rows at partitions 0:64 and h1 at 64:128 of block 0 (so the
        # h0+h1 pair contracts as one K=128 matmul); h2 in block 1.
        wp_sb = P.tile([128, 2 * D], BF16, tag="wp")
        nc.sync.dma_start(wp_sb[:], wp[:])
        ones_f = P.tile([128, HD], F32, tag="ones_f")
        nc.vector.memset(ones_f[:], 1.0)
        ones = P.tile([128, HD], F32R, tag="ones")
        nc.vector.tensor_copy(ones[:], ones_f[:])



        qT = P.tile([128, 2 * Q], BF16, tag="qT")       # q^T: rows=[h0|h1], [h2]
        kT = P.tile([128, 2 * KP], BF16, tag="kT")      # k^T subsampled
        qT_dup = P.tile([128, 2 * Q], BF16, tag="qTd")  # row-halves swapped
        kT_dup = P.tile([128, 2 * KP], BF16, tag="kTd")
        # v + ones col per (head, ktile) slot, padded to 128 weight columns
        # (full-width LDWEIGHTS pipelines with the PV matmul stream; cols
        # 65:128 stay zero so the extra output rows are zeros)
        vv = P.tile([128, HPG * NKT * 128], BF16, tag="v")
        # attn out: block 0 rows 0:64 = h0, rows 64:128 = h1; block 1 = h2
        attn = P.tile([128, 2 * Q], BF16, tag="attn")

        # zero the pad columns, ones columns of the v tiles
        nc.vector.memset(vv[:], 0.0)
        nc.vector.memset(vv[:, 64 : HPG * NKT * 128 : 128], 1.0)

        # ---- A: projection emitters (pipelined into the B loop) ----------
        def qproj_group(c, m):
            msz = 128 if m == 0 else 64
            pool, tag = (SCPS, "sc") if m == 0 else (NUMPS, "num")
            w = 1536 if m == 0 else 512
            ps = pool.tile([128, w], F32, tag=tag, name=f"qp{c}_{m}")
            for kt in range(NDK):
                nc.tensor.matmul(
                    ps[0:msz, 0:CH],
                    wq_sb[:, kt * GD + m * 128 : kt * GD + m * 128 + msz],
                    xt[:, kt * Q + c * CH : kt * Q + (c + 1) * CH],
                    start=(kt == 0),
                    stop=(kt == NDK - 1),
                )
            nc.vector.tensor_copy(
                qT[0:msz, m * Q + c * CH : m * Q + (c + 1) * CH],
                ps[0:msz, 0:CH],
            )

        kchunks = [(0, 448), (448, 448), (896, 448), (1344, 224)]

        def kproj_group(m, ci):
            msz = 128 if m == 0 else 64
            pool, tag = (SCPS, "sc") if m == 0 else (NUMPS, "num")
            w = 1536 if m == 0 else 512
            c0, csz = kchunks[ci]
            ps = pool.tile([128, w], F32, tag=tag, name=f"kp{ci}_{m}")
            for kt in range(NDK):
                base = kt * Q
                nc.tensor.matmul(
                    ps[0:msz, 0:csz],
                    wk_sb[:, kt * GD + m * 128 : kt * GD + m * 128 + msz],
                    xt[:, base + 2 * c0 : base + 2 * (c0 + csz) : 2],
                    start=(kt == 0),
                    stop=(kt == NDK - 1),
                )
            nc.vector.tensor_copy(
                kT[0:msz, m * KP + c0 : m * KP + c0 + csz],
                ps[0:msz, 0:csz],
            )

        def vproj_group(kt_m):
            msz = _ksize(kt_m)
            ps = NUMPS.tile([128, 512], F32, tag="num", name=f"vp{kt_m}")
            for kt in range(NDK):
                base = kt * Q + 2 * (kt_m * 128)
                nc.tensor.matmul(
                    ps[0:msz, 0:GD],
                    xt[:, base : base + 2 * msz : 2],
                    wv_sb[:, kt * GD : (kt + 1) * GD],
                    start=(kt == 0),
                    stop=(kt == NDK - 1),
                )
            for h in range(HPG):
                slot = (h * NKT + kt_m) * 128
                nc.vector.tensor_copy(
                    vv[0:msz, slot : slot + 64],
                    ps[0:msz, h * HD : (h + 1) * HD],
                )

        # upfront: qproj paced by the chunk-major x DMA arrival, then kproj
        # (which needs even columns of every chunk); vproj rides iteration 0
        for c in range(NCH):
            qproj_group(c, 0)
            qproj_group(c, 1)
        for ci in range(4):
            kproj_group(0, ci)
            kproj_group(1, ci)
        # row-swapped duplicates of qT/kT so score matmuls can alternate
        # PE row groups by k-tile parity; queued behind the x loads
        nc.sync.dma_start(qT_dup[HD:128, 0:Q], qT[0:HD, 0:Q])
        nc.scalar.dma_start(qT_dup[0:HD, 0:Q], qT[HD:128, 0:Q])
        nc.sync.dma_start(qT_dup[HD:128, Q : 2 * Q], qT[0:HD, Q : 2 * Q])
        nc.scalar.dma_start(kT_dup[HD:128, 0:KP], kT[0:HD, 0:KP])
        nc.sync.dma_start(kT_dup[0:HD, 0:KP], kT[HD:128, 0:KP])
        nc.scalar.dma_start(kT_dup[HD:128, KP : 2 * KP], kT[0:HD, KP : 2 * KP])

        # ---- B/C: attention, software-pipelined by one q-chunk -----------
        def emit_score_group(h, c, es, grp):
            blk, pb = _head_pos(h)
            if True:
                ng = len(grp)
                scp = SCPS.tile([128, 1536], F32, tag="sc")
                pmax = _ksize(grp[-1])
                for j, kt in enumerate(grp):
                    ksz = _ksize(kt)
                    if kt % 2 == 0:
                        sk, sq, base = kT, qT, pb
                    else:
                        sk, sq, base = kT_dup, qT_dup, HD - pb
                    nc.tensor.matmul(
                        scp[0:ksz, j * 512 : j * 512 + CH],
                        sk[base : base + HD, blk * KP + kt * 128 : blk * KP + kt * 128 + ksz],
                        sq[base : base + HD, blk * Q + c * CH : blk * Q + (c + 1) * CH],
                        start=True,
                        stop=True,
                        tile_position=(base, 0),
                    )
                src = scp[0:pmax, 0 : ng * 512].rearrange(
                    "p (a b) -> p a b", b=512
                )[:, :, 0:CH] if ng > 1 else scp[0:pmax, 0:CH]
                dst = es[
                    0:pmax, grp[0] * CH : (grp[-1] + 1) * CH
                ].rearrange("p (a b) -> p a b", b=CH) if ng > 1 else es[
                    0:pmax, grp[0] * CH : grp[0] * CH + CH
                ]
                nc.scalar.activation(dst, src, AF.Exp, scale=SC)

        def pv_part(h, c, es):
            num = NUMPS.tile([128, 512], F32, tag="num")
            for kt in range(NKT):
                ksz = _ksize(kt)
                slot = (h * NKT + kt) * 128
                nc.tensor.matmul(
                    num[0:128, 0:CH],
                    vv[0:ksz, slot : slot + 128],
                    es[0:ksz, kt * CH : (kt + 1) * CH],
                    start=(kt == 0),
                    stop=(kt == NKT - 1),
                )
            den = INV.tile([128, CH], F32, tag="den")
            nc.vector.tensor_copy(den[0:1, :], num[64:65, 0:CH])
            nsb = INV.tile([128, CH], F32, tag="nsb", bufs=3)
            nc.vector.tensor_copy(nsb[0:64, :], num[0:64, 0:CH])
            inv = INV.tile([128, CH], F32, tag="inv", bufs=3)
            nc.vector.reciprocal_approx_fast(inv[0:1, :], den[0:1, :])
            return num, nsb, inv

        def rep_mult(h, c, num, nsb, inv):
            # broadcast 1/den across 64 rows on the (idle) gpsimd engine,
            # then multiply with the staged PV rows
            binv = INV.tile([128, CH], F32, tag="binv", bufs=3)
            nc.gpsimd.partition_broadcast(binv[0:64, :], inv[0:1, 0:CH])
            if h == 0:
                dst = attn[0:64, c * CH : (c + 1) * CH]
            elif h == 1:
                dst = attn[64:128, c * CH : (c + 1) * CH]
            else:
                dst = attn[0:64, Q + c * CH : Q + (c + 1) * CH]
            nc.vector.tensor_tensor(
                dst, nsb[0:64, :], binv[0:64, :], op=mybir.AluOpType.mult
            )

        def proj_one(c, m):
            if True:
                pp = SCPS.tile([128, 1536], F32, tag="sc", name=f"pj{m}")
                nc.tensor.matmul(
                    pp[0:128, 0:CH],
                    wp_sb[0:128, m * 128 : (m + 1) * 128],
                    attn[0:128, c * CH : (c + 1) * CH],
                    start=True,
                    stop=False,
                )
                nc.tensor.matmul(
                    pp[0:128, 0:CH],
                    wp_sb[0:HD, D + m * 128 : D + (m + 1) * 128],
                    attn[0:HD, Q + c * CH : Q + (c + 1) * CH],
                    start=False,
                    stop=True,
                )
                ot = OT.tile([128, CH], F32, tag="ot")
                nc.vector.tensor_copy(ot[:], pp[0:128, 0:CH])
                nc.sync.dma_start(
                    out[m * 128 : (m + 1) * 128, c * CH : (c + 1) * CH], ot[:]
                )

        def mk_qp(qc, qm):
            def f():
                qproj_group(qc, qm)
            return f

        def mk_vp(kt_m):
            def f():
                vproj_group(kt_m)
            return f

        pv_queue = []
        for c in range(NCH):
            es = [ES.tile([128, NKT * CH], BF16, tag="es", name=f"es{c}_{h}")
                  for h in range(HPG)]
            if pv_queue:
                pc = pv_queue[0][1]
                state = {}

                def mk_pv(h, ppc, pes):
                    def f():
                        state[h] = pv_part(h, ppc, pes)
                    return f

                def mk_rep(h, ppc):
                    def f():
                        num, nsb, invr = state[h]
                        rep_mult(h, ppc, num, nsb, invr)
                    return f

                def mk_proj(m, ppc):
                    def f():
                        proj_one(ppc, m)
                    return f

                def mk_seq(*fns):
                    def f():
                        for g in fns:
                            g()
                    return f

                f_pv = [mk_pv(h, ppc, pes) for h, ppc, pes in pv_queue]
                f_rep = [mk_rep(h, ppc) for h, ppc, pes in pv_queue]
                f_pj = [mk_proj(m, pc) for m in range(NDK)]
                fillers = [
                    f_pv[0], f_pv[1], f_pv[2],
                    f_rep[0], f_rep[1], f_rep[2],
                    f_pj[0], f_pj[1], f_pj[2], f_pj[3],
                    mk_seq(f_pj[4], f_pj[5]),
                ]
            else:
                # iteration 0: vproj rides as fillers
                fillers = [mk_vp(k) for k in range(NKT)]
            # fillers slot between score groups to keep all engines fed;
            # last chunk emits h2 first so the epilogue's pv(h2) doesn't
            # wait on the tail exp groups
            horder = (2, 0, 1) if c == NCH - 1 else (0, 1, 2)
            fi = 0
            gi = 0
            for h in horder:
                for grp in EXP_GROUPS:
                    emit_score_group(h, c, es[h], grp)
                    gi += 1
                    if fi < len(fillers):
                        fillers[fi]()
                        fi += 1
            while fi < len(fillers):
                fillers[fi]()
                fi += 1
            pv_queue = [(0, c, es[0]), (1, c, es[1]), (2, c, es[2])]
        pv_queue = [pv_queue[2], pv_queue[0], pv_queue[1]]
        parts = [pv_part(h, ppc, pes) for h, ppc, pes in pv_queue]
        for (h, ppc, pes), (num, nsb, invr) in zip(pv_queue, parts):
            rep_mult(h, ppc, num, nsb, invr)
        for m in range(NDK):
            proj_one(NCH - 1, m)


def _get_nc():
    if "nc" not in _CACHE:
        _CACHE["nc"] = _build_nc()
    return _CACHE["nc"]


def kernel(x, W_qkv, W_proj, b_proj):
    nc = _get_nc()
    xTs = [
        np.ascontiguousarray(
            x[n].reshape(Q, D).astype(BFNP).T
        )
        for n in range(N)
    ]
    def _wlay(w):
        # [768, 192] -> [128, 6*192] in the SBUF layout (kt-major columns)
        return np.ascontiguousarray(
            w.reshape(NDK, 128, GD).transpose(1, 0, 2).reshape(128, NDK * GD)
        )

    wqs, wks, wvs, wps = [], [], [], []
    for g in range(4):
        c0 = g * GD
        wqs.append(_wlay(W_qkv[:, c0 : c0 + GD].astype(BFNP)))
        wks.append(_wlay(W_qkv[:, D + c0 : D + c0 + GD].astype(BFNP)))
        wvs.append(_wlay(W_qkv[:, 2 * D + c0 : 2 * D + c0 + GD].astype(BFNP)))
        wpg = W_proj[c0 : c0 + GD, :].astype(BFNP)
        wpl = np.zeros((128, 2 * D), BFNP)
        wpl[0:HD, 0:D] = wpg[0:HD]
        wpl[HD:128, 0:D] = wpg[HD : 2 * HD]
        wpl[0:HD, D : 2 * D] = wpg[2 * HD : 3 * HD]
        wps.append(wpl)
    in_maps = [
        {"xT": xTs[c // 4], "wq": wqs[c % 4], "wk": wks[c % 4],
         "wv": wvs[c % 4], "wp": wps[c % 4]}
        for c in range(N_CORES)
    ]
    res = run_bass_kernel_spmd(nc, in_maps, list(range(N_CORES)), trace=TRACE)
    if TRACE:
        LAST_RESULTS["exec_time_ns"] = res.exec_time_ns
        LAST_RESULTS["mean_exec_time_ns"] = res.mean_exec_time_ns
    out = np.empty((N, T, S, D), np.float32)
    for n in range(N):
        acc = res.results[4 * n]["out"]
        for g in range(1, 4):
            acc = acc + res.results[4 * n + g]["out"]
        out[n] = (acc.T + b_proj).reshape(T, S, D)
    return out



# revision 36
# speedup vs baseline: 1.0385x; 1.0385x over previous
"""Trainium2 Bass kernel for nn_AttentionSubsample (8-core SPMD).

Sharding: batch N=2 x 4 head-groups (3 heads each) -> 8 cores, no
collectives.  Each core computes q/k/v projections for its head group
(K/V on the stride-2 subsampled positions only), per-head attention with
softmax folded as exp -> denominator via an appended ones-column in V ->
divide, and its partial output projection in transposed layout.  The
host sums the 4 per-batch partials and adds the bias.

Layout notes:
 - The spatial stride-2 subsample of K/V equals taking even rows of the
   flattened [3136, 768] batch (196 is even), i.e. even columns of x^T.
 - All device matmuls run in bf16 (fp32 PSUM accumulation).
"""

import sys

for _p in ("/opt/trn_rl_repo",):
    if _p not in sys.path:
        sys.path.insert(0, _p)

import numpy as np
import ml_dtypes

import concourse.bass as bass  # noqa: F401  (registers engines)
import concourse.tile as tile
from concourse import bacc, mybir
from concourse.bass_utils import run_bass_kernel_spmd

BFNP = ml_dtypes.bfloat16
F32 = mybir.dt.float32
F32R = mybir.dt.float32r
BF16 = mybir.dt.bfloat16
AF = mybir.ActivationFunctionType

N, T, S, D = 2, 16, 196, 768
H, HD = 12, 64
Q = T * S              # 3136 query positions per batch
KP = T * (S // 2)      # 1568 subsampled key positions
HPG = 3                # heads per group (12 heads / 4 groups)
GD = HPG * HD          # 192 channels per head group
SC = (D // H) ** -0.5  # 0.125 attention scale
CH = 448               # q-chunk size (3136 = 7 * 448)
NCH = Q // CH          # 7
NKT = 13               # k tiles: 12 * 128 + 32
KTL = 32               # last k-tile height
NDK = D // 128         # 6 contraction tiles for the projections
N_CORES = 8

# exp groups over k-tiles: PSUM scores tile holds 3 banks (512-aligned)
EXP_GROUPS = [(0, 1, 2), (3, 4, 5), (6, 7, 8), (9, 10, 11), (12,)]

TRACE = False          # test.py flips this for profiled runs
LAST_RESULTS = {}      # exec_time_ns etc. stashed here on traced runs

_CACHE = {}


def _ksize(kt):
    return 128 if kt < NKT - 1 else KTL


def _head_pos(h):
    """(block, partition base) of head h inside the 2-block qT/kT tiles."""
    return (0, 0) if h == 0 else ((0, 64) if h == 1 else (1, 0))


def _build_nc():
    nc = bacc.Bacc(
        "TRN2", target_bir_lowering=False, debug=False, num_devices=N_CORES
    )
    # weights arrive pre-rearranged from the host in their exact SBUF
    # layouts so the loads are single contiguous DMAs
    xT = nc.dram_tensor("xT", [D, Q], BF16, kind="ExternalInput").ap()
    wq = nc.dram_tensor("wq", [128, NDK * GD], BF16, kind="ExternalInput").ap()
    wk = nc.dram_tensor("wk", [128, NDK * GD], BF16, kind="ExternalInput").ap()
    wv = nc.dram_tensor("wv", [128, NDK * GD], BF16, kind="ExternalInput").ap()
    wp = nc.dram_tensor("wp", [128, 2 * D], BF16, kind="ExternalInput").ap()
    out = nc.dram_tensor("out", [D, Q], F32, kind="ExternalOutput").ap()

    with tile.TileContext(nc) as tc:
        _body(tc, xT, wq, wk, wv, wp, out)
    nc.compile()
    return nc


def _body(tc, xT, wq, wk, wv, wp, out):
    nc = tc.nc
    with (
        tc.tile_pool(name="persist", bufs=1) as P,
        tc.tile_pool(name="es", bufs=6) as ES,
        tc.tile_pool(name="inv", bufs=2) as INV,
        tc.tile_pool(name="ot", bufs=3) as OT,
        tc.tile_pool(name="scps", bufs=2, space="PSUM") as SCPS,
        tc.tile_pool(name="numps", bufs=2, space="PSUM") as NUMPS,
    ):
        # PE warm-up scratch first so its memsets clear the vector queue
        # before the big vv memsets
        warm_w = P.tile([128, 128], BF16, tag="warm_w")
        nc.vector.memset(warm_w[:], 0.0)
        warm_x = P.tile([128, 448], BF16, tag="warm_x")
        nc.vector.memset(warm_x[:], 0.0)
        for wi in range(14):
            wps = SCPS.tile([128, 512], F32, tag="sc", name=f"warm{wi}")
            nc.tensor.matmul(
                wps[0:128, 0:448], warm_w[:], warm_x[:], start=True, stop=True
            )

        # ---- persistent SBUF tensors -------------------------------------
        wq_sb = P.tile([128, NDK * GD], BF16, tag="wq")
        nc.scalar.dma_start(wq_sb[:], wq[:])
        # x loaded chunk-major across both DMA queues so qproj c=0 starts early
        xt = P.tile([128, NDK * Q], BF16, tag="xt")
        _dmae = (nc.sync, nc.scalar)
        for c in range(NCH):
            for kt in range(NDK):
                _dmae[kt % 2].dma_start(
                    xt[:, kt * Q + c * CH : kt * Q + (c + 1) * CH],
                    xT[kt * 128 : (kt + 1) * 128, c * CH : (c + 1) * CH],
                )
        wk_sb = P.tile([128, NDK * GD], BF16, tag="wk")
        nc.sync.dma_start(wk_sb[:], wk[:])
        wv_sb = P.tile([128, NDK * GD], BF16, tag="wv")
        nc.sync.dma_start(wv_sb[:], wv[:])
        # wp: h0 rows at partitions 0:64 and h1 at 64:128 of block 0 (so the
        # h0+h1 pair contracts as one K=128 matmul); h2 in block 1.
        wp_sb = P.tile([128, 2 * D], BF16, tag="wp")
        nc.sync.dma_start(wp_sb[:], wp[:])
        ones_f = P.tile([128, HD], F32, tag="ones_f")
        nc.vector.memset(ones_f[:], 1.0)
        ones = P.tile([128, HD], F32R, tag="ones")
        nc.vector.tensor_copy(ones[:], ones_f[:])



        qT = P.tile([128, 2 * Q], BF16, tag="qT")       # q^T: rows=[h0|h1], [h2]
        kT = P.tile([128, 2 * KP], BF16, tag="kT")      # k^T subsampled
        qT_dup = P.tile([128, 2 * Q], BF16, tag="qTd")  # row-halves swapped
        kT_dup = P.tile([128, 2 * KP], BF16, tag="kTd")
        # v + ones col per (head, ktile) slot, padded to 128 weight columns
        # (full-width LDWEIGHTS pipelines with the PV matmul stream; cols
        # 65:128 stay zero so the extra output rows are zeros)
        vv = P.tile([128, HPG * NKT * 128], BF16, tag="v")
        # attn out: block 0 rows 0:64 = h0, rows 64:128 = h1; block 1 = h2
        attn = P.tile([128, 2 * Q], BF16, tag="attn")

        # zero the pad columns, ones columns of the v tiles
        nc.vector.memset(vv[:], 0.0)
        nc.vector.memset(vv[:, 64 : HPG * NKT * 128 : 128], 1.0)

        # ---- A: projection emitters (pipelined into the B loop) ----------
        def qproj_group(c, m):
            msz = 128 if m == 0 else 64
            pool, tag = (SCPS, "sc") if m == 0 else (NUMPS, "num")
            w = 1536 if m == 0 else 512
            ps = pool.tile([128, w], F32, tag=tag, name=f"qp{c}_{m}")
            for kt in range(NDK):
                nc.tensor.matmul(
                    ps[0:msz, 0:CH],
                    wq_sb[:, kt * GD + m * 128 : kt * GD + m * 128 + msz],
                    xt[:, kt * Q + c * CH : kt * Q + (c + 1) * CH],
                    start=(kt == 0),
                    stop=(kt == NDK - 1),
                )
            nc.vector.tensor_copy(
                qT[0:msz, m * Q + c * CH : m * Q + (c + 1) * CH],
                ps[0:msz, 0:CH],
            )

        kchunks = [(0, 448), (448, 448), (896, 448), (1344, 224)]

        def kproj_group(m, ci):
            msz = 128 if m == 0 else 64
            pool, tag = (SCPS, "sc") if m == 0 else (NUMPS, "num")
            w = 1536 if m == 0 else 512
            c0, csz = kchunks[ci]
            ps = pool.tile([128, w], F32, tag=tag, name=f"kp{ci}_{m}")
            for kt in range(NDK):
                base = kt * Q
                nc.tensor.matmul(
                    ps[0:msz, 0:csz],
                    wk_sb[:, kt * GD + m * 128 : kt * GD + m * 128 + msz],
                    xt[:, base + 2 * c0 : base + 2 * (c0 + csz) : 2],
                    start=(kt == 0),
                    stop=(kt == NDK - 1),
                )
            nc.vector.tensor_copy(
                kT[0:msz, m * KP + c0 : m * KP + c0 + csz],
                ps[0:msz, 0:csz],
            )

        def vproj_group(kt_m):
            msz = _ksize(kt_m)
            ps = NUMPS.tile([128, 512], F32, tag="num", name=f"vp{kt_m}")
            for kt in range(NDK):
                base = kt * Q + 2 * (kt_m * 128)
                nc.tensor.matmul(
                    ps[0:msz, 0:GD],
                    xt[:, base : base + 2 * msz : 2],
                    wv_sb[:, kt * GD : (kt + 1) * GD],
                    start=(kt == 0),
                    stop=(kt == NDK - 1),
                )
            for h in range(HPG):
                slot = (h * NKT + kt_m) * 128
                nc.vector.tensor_copy(
                    vv[0:msz, slot : slot + 64],
                    ps[0:msz, h * HD : (h + 1) * HD],
                )

        # upfront: qproj paced by the chunk-major x DMA arrival, then kproj
        # (which needs even columns of every chunk); vproj rides iteration 0
        for c in range(NCH):
            qproj_group(c, 0)
            qproj_group(c, 1)
        for ci in range(4):
            kproj_group(0, ci)
            kproj_group(1, ci)
        # row-swapped duplicates of qT/kT so score matmuls can alternate
        # PE row groups by k-tile parity; queued behind the x loads
        nc.sync.dma_start(qT_dup[HD:128, 0:Q], qT[0:HD, 0:Q])
        nc.scalar.dma_start(qT_dup[0:HD, 0:Q], qT[HD:128, 0:Q])
        nc.sync.dma_start(qT_dup[HD:128, Q : 2 * Q], qT[0:HD, Q : 2 * Q])
        nc.scalar.dma_start(kT_dup[HD:128, 0:KP], kT[0:HD, 0:KP])
        nc.sync.dma_start(kT_dup[0:HD, 0:KP], kT[HD:128, 0:KP])
        nc.scalar.dma_start(kT_dup[HD:128, KP : 2 * KP], kT[0:HD, KP : 2 * KP])

        # ---- B/C: attention, software-pipelined by one q-chunk -----------
        def emit_score_group(h, c, es, grp):
            blk, pb = _head_pos(h)
            if True:
                ng = len(grp)
                scp = SCPS.tile([128, 1536], F32, tag="sc")
                pmax = _ksize(grp[-1])
                for j, kt in enumerate(grp):
                    ksz = _ksize(kt)
                    if kt % 2 == 0:
                        sk, sq, base = kT, qT, pb
                    else:
                        sk, sq, base = kT_dup, qT_dup, HD - pb
                    nc.tensor.matmul(
                        scp[0:ksz, j * 512 : j * 512 + CH],
                        sk[base : base + HD, blk * KP + kt * 128 : blk * KP + kt * 128 + ksz],
                        sq[base : base + HD, blk * Q + c * CH : blk * Q + (c + 1) * CH],
                        start=True,
                        stop=True,
                        tile_position=(base, 0),
                    )
                src = scp[0:pmax, 0 : ng * 512].rearrange(
                    "p (a b) -> p a b", b=512
                )[:, :, 0:CH] if ng > 1 else scp[0:pmax, 0:CH]
                dst = es[
                    0:pmax, grp[0] * CH : (grp[-1] + 1) * CH
                ].rearrange("p (a b) -> p a b", b=CH) if ng > 1 else es[
                    0:pmax, grp[0] * CH : grp[0] * CH + CH
                ]
                nc.scalar.activation(dst, src, AF.Exp, scale=SC)

        def pv_part(h, c, es):
            num = NUMPS.tile([128, 512], F32, tag="num")
            for kt in range(NKT):
                ksz = _ksize(kt)
                slot = (h * NKT + kt) * 128
                nc.tensor.matmul(
                    num[0:128, 0:CH],
                    vv[0:ksz, slot : slot + 128],
                    es[0:ksz, kt * CH : (kt + 1) * CH],
                    start=(kt == 0),
                    stop=(kt == NKT - 1),
                )
            den = INV.tile([128, CH], F32, tag="den")
            nc.vector.tensor_copy(den[0:1, :], num[64:65, 0:CH])
            nsb = INV.tile([128, CH], F32, tag="nsb", bufs=3)
            nc.vector.tensor_copy(nsb[0:64, :], num[0:64, 0:CH])
            inv = INV.tile([128, CH], F32, tag="inv", bufs=3)
            nc.vector.reciprocal_approx_fast(inv[0:1, :], den[0:1, :])
            return num, nsb, inv

        def rep_mult(h, c, num, nsb, inv):
            # broadcast 1/den across 64 rows on the (idle) gpsimd engine,
            # then multiply with the staged PV rows
            binv = INV.tile([128, CH], F32, tag="binv", bufs=3)
            nc.gpsimd.partition_broadcast(binv[0:64, :], inv[0:1, 0:CH])
            if h == 0:
                dst = attn[0:64, c * CH : (c + 1) * CH]
            elif h == 1:
                dst = attn[64:128, c * CH : (c + 1) * CH]
            else:
                dst = attn[0:64, Q + c * CH : Q + (c + 1) * CH]
            nc.vector.tensor_tensor(
                dst, nsb[0:64, :], binv[0:64, :], op=mybir.AluOpType.mult
            )
            if h == 2:
                # duplicate h2's attn rows onto partitions 64:128 so the
                # K=64 oproj matmuls can row-pair across m tiles
                nc.sync.dma_start(
                    attn[64:128, Q + c * CH : Q + (c + 1) * CH],
                    attn[0:64, Q + c * CH : Q + (c + 1) * CH],
                )

        def proj_pair(c, m):
            # output projection for m tiles m and m+1: two full-K (h0|h1)
            # matmuls plus the two K=64 h2 parts row-paired so they run
            # concurrently on different PE row groups
            pp = SCPS.tile([128, 1536], F32, tag="sc", name=f"pj{m}")
            for j in (0, 1):
                nc.tensor.matmul(
                    pp[0:128, j * 512 : j * 512 + CH],
                    wp_sb[0:128, (m + j) * 128 : (m + j + 1) * 128],
                    attn[0:128, c * CH : (c + 1) * CH],
                    start=True,
                    stop=False,
                )
            nc.tensor.matmul(
                pp[0:128, 0:CH],
                wp_sb[0:HD, D + m * 128 : D + (m + 1) * 128],
                attn[0:HD, Q + c * CH : Q + (c + 1) * CH],
                start=False,
                stop=True,
                tile_position=(0, 0),
            )
            nc.tensor.matmul(
                pp[0:128, 512 : 512 + CH],
                wp_sb[HD:128, D + (m + 1) * 128 : D + (m + 2) * 128],
                attn[HD:128, Q + c * CH : Q + (c + 1) * CH],
                start=False,
                stop=True,
                tile_position=(64, 0),
            )
            ot = OT.tile([128, 2 * CH], F32, tag="ot")
            nc.vector.tensor_copy(
                ot[:].rearrange("p (a b) -> p a b", a=2),
                pp[0:128, 0:1024].rearrange("p (a b) -> p a b", b=512)[:, :, 0:CH],
            )
            for j in (0, 1):
                nc.sync.dma_start(
                    out[(m + j) * 128 : (m + j + 1) * 128, c * CH : (c + 1) * CH],
                    ot[:, j * CH : (j + 1) * CH],
                )

        def mk_qp(qc, qm):
            def f():
                qproj_group(qc, qm)
            return f

        def mk_vp(kt_m):
            def f():
                vproj_group(kt_m)
            return f

        pv_queue = []
        for c in range(NCH):
            es = [ES.tile([128, NKT * CH], BF16, tag="es", name=f"es{c}_{h}")
                  for h in range(HPG)]
            if pv_queue:
                pc = pv_queue[0][1]
                state = {}

                def mk_pv(h, ppc, pes):
                    def f():
                        state[h] = pv_part(h, ppc, pes)
                    return f

                def mk_rep(h, ppc):
                    def f():
                        num, nsb, invr = state[h]
                        rep_mult(h, ppc, num, nsb, invr)
                    return f

                def mk_proj(m, ppc):
                    def f():
                        proj_pair(ppc, m)
                    return f

                f_pv = [mk_pv(h, ppc, pes) for h, ppc, pes in pv_queue]
                f_rep = [mk_rep(h, ppc) for h, ppc, pes in pv_queue]
                f_pj = [mk_proj(m, pc) for m in (0, 2, 4)]
                fillers = [
                    f_pv[0], f_pv[1], f_pv[2],
                    f_rep[0], f_rep[1], f_rep[2],
                    f_pj[0], f_pj[1], f_pj[2],
                ]
            else:
                # iteration 0: vproj rides as fillers
                fillers = [mk_vp(k) for k in range(NKT)]
            # fillers slot between score groups to keep all engines fed;
            # last chunk emits h2 first so the epilogue's pv(h2) doesn't
            # wait on the tail exp groups
            horder = (2, 0, 1) if c == NCH - 1 else (0, 1, 2)
            fi = 0
            gi = 0
            for h in horder:
                for grp in EXP_GROUPS:
                    emit_score_group(h, c, es[h], grp)
                    gi += 1
                    if fi < len(fillers):
                        fillers[fi]()
                        fi += 1
            while fi < len(fillers):
                fillers[fi]()
                fi += 1
            pv_queue = [(0, c, es[0]), (1, c, es[1]), (2, c, es[2])]
        pv_queue = [pv_queue[2], pv_queue[0], pv_queue[1]]
        parts = [pv_part(h, ppc, pes) for h, ppc, pes in pv_queue]
        for (h, ppc, pes), (num, nsb, invr) in zip(pv_queue, parts):
            rep_mult(h, ppc, num, nsb, invr)
        for m in (0, 2, 4):
            proj_pair(NCH - 1, m)


def _get_nc():
    if "nc" not in _CACHE:
        _CACHE["nc"] = _build_nc()
    return _CACHE["nc"]


def kernel(x, W_qkv, W_proj, b_proj):
    nc = _get_nc()
    xTs = [
        np.ascontiguousarray(
            x[n].reshape(Q, D).astype(BFNP).T
        )
        for n in range(N)
    ]
    def _wlay(w):
        # [768, 192] -> [128, 6*192] in the SBUF layout (kt-major columns)
        return np.ascontiguousarray(
            w.reshape(NDK, 128, GD).transpose(1, 0, 2).reshape(128, NDK * GD)
        )

    wqs, wks, wvs, wps = [], [], [], []
    for g in range(4):
        c0 = g * GD
        wqs.append(_wlay(W_qkv[:, c0 : c0 + GD].astype(BFNP)))
        wks.append(_wlay(W_qkv[:, D + c0 : D + c0 + GD].astype(BFNP)))
        wvs.append(_wlay(W_qkv[:, 2 * D + c0 : 2 * D + c0 + GD].astype(BFNP)))
        wpg = W_proj[c0 : c0 + GD, :].astype(BFNP)
        wpl = np.zeros((128, 2 * D), BFNP)
        wpl[0:HD, 0:D] = wpg[0:HD]
        wpl[HD:128, 0:D] = wpg[HD : 2 * HD]
        wpl[0:HD, D : 2 * D] = wpg[2 * HD : 3 * HD]
        # second copy of the h2 rows on partitions 64:128 so the K=64
        # oproj matmuls can row-pair across adjacent m tiles
        wpl[HD:128, D : 2 * D] = wpg[2 * HD : 3 * HD]
        wps.append(wpl)
    in_maps = [
        {"xT": xTs[c // 4], "wq": wqs[c % 4], "wk": wks[c % 4],
         "wv": wvs[c % 4], "wp": wps[c % 4]}
        for c in range(N_CORES)
    ]
    res = run_bass_kernel_spmd(nc, in_maps, list(range(N_CORES)), trace=TRACE)
    if TRACE:
        LAST_RESULTS["exec_time_ns"] = res.exec_time_ns
        LAST_RESULTS["mean_exec_time_ns"] = res.mean_exec_time_ns
    out = np.empty((N, T, S, D), np.float32)
    for n in range(N):
        acc = res.results[4 * n]["out"]
        for g in range(1, 4):
            acc = acc + res.results[4 * n + g]["out"]
        out[n] = (acc.T + b_proj).reshape(T, S, D)
    return out

